# revision 1
# baseline (speedup 1.0000x reference)
"""Trainium2 Bass kernel for the Householder-chain problem.

Computes y = x @ Q.T where Q = M_0 @ M_1 @ ... @ M_{N-1} is a product of
N=514 Householder reflections M_i = I - 2 v_i v_i^T / (v_i^T v_i + eps)
over S=512 dims, and x is [65536, 512].

Math: since each M_i is symmetric, Q.T = M_{N-1} @ ... @ M_0 =: A, and the
product collapses via the compact-WY representation with natural column
order:  A = I - V T V^T  where V = [v_0 ... v_{N-1}] (S x N) and
T^{-1} = R = stril(V^T V) + diag((||v_i||^2 + eps)/2)   (lower triangular).

On device (replicated on each of 8 cores, since it is tiny):
  G = V^T V; R's 128x128 diagonal blocks are inverted by Newton iteration
  (X <- X(2I - R X), exact in ceil(log2(128)) = 7 steps for triangular R);
  off-diagonal blocks by block back-substitution; then
  A = I - (V T)(V^T) via two small matmul chains.  All in fp32 (the PE's
  full-precision path) - A must be accurate to ~1e-6.
N is zero-padded 514 -> 640 with unit diagonal entries in R for pad
columns, which leaves A unchanged.

Main work: y = x @ A, data-parallel over the 65536 rows across 8 cores
(8192 rows/core) - memory-bound streaming matmul.  It runs in the PE's
float32r mode (fp32 storage, RNE-to-11-mantissa-bit rounding inside the
matmul, 4x faster than the fp32 path): measured end-to-end relative error
~1.5e-4.  Set COMPENSATED=True for a 3-term error-compensated variant
(x and A split into 11-bit head + tail; y = xh Ah + xl Ah + xh Al) that
restores ~1.5e-6 relative error at 3x the PE cost.

x is transposed on the host once so the contraction dim (s) lands on SBUF
partitions.
"""

from contextlib import ExitStack

import numpy as np

import bass_rust
import concourse.bass as bass
import concourse.mybir as mybir
import concourse.tile as tile
from concourse.bass_utils import run_bass_kernel_spmd
from concourse.masks import make_identity, make_upper_triangular
from concourse.vector_clock import ScopedClock

FP = mybir.dt.float32
FPR = mybir.dt.float32r
U32 = mybir.dt.uint32
AX = mybir.AxisListType
OP = mybir.AluOpType

S = 512           # feature dim
NV = 514          # number of householder vectors
NP = 640          # padded vector count (5 * 128)
NB = NP // 128    # 5 blocks
B = 65536         # batch rows
NCORES = 8
BPC = B // NCORES  # 8192 rows per core
EPS = 1e-16
CW = 1024         # main-loop x chunk width (batch cols per chunk)
NEWTON_ITERS = 7
COMPENSATED = False  # 3-term f32r error compensation in the main matmul
HEAD_MASK = 0xFFFFF000  # keep sign+exp+11 mantissa bits (= f32r precision)


# ---------------------------------------------------------------------------
# walrus CTRL instructions accept at most 4 sem waits, and this Tile
# version puts the whole global-clock wait set on the single tail drain.
# Spread the waits over preceding SP nops (1 wait each, conservatively).
def _patched_drain_and_barrier(self, tick_clock, wait_clock):
    pre_nops = [self.nc.sync.nop() for _ in range(30)]
    drain_inst = self.nc.sync.drain()
    wait_clock.add_sem_waits(
        drain_inst.ins, ScopedClock({None: tick_clock.global_clock})
    )
    si = drain_inst.ins.sync_info
    waits = list(si.on_wait) if si is not None and si.on_wait else []
    if len(waits) > 1:
        assert len(waits) - 1 <= len(pre_nops), "too many drain waits"
        for nop, w in zip(pre_nops, waits[:-1]):
            nop.ins.sync_info = bass_rust.SyncInfo(on_wait=[w], on_update=[])
        upd = list(si.on_update) if si.on_update else []
        drain_inst.ins.sync_info = bass_rust.SyncInfo(
            on_wait=[waits[-1]], on_update=upd)

    self.nc.all_engine_barrier()
    assert self.sems is not None
    popped = self.nc._tile_sem_poison_stack.pop()
    assert popped is self._sem_poison
    self.nc.clear_and_free_semaphores(list(self.sems.allocated().values()))
    self.nc.all_engine_barrier()


tile.TileContext._drain_and_barrier = _patched_drain_and_barrier


def _split_excess_waits(nc, max_waits=1):
    """This walrus build accepts very few sem waits per instruction (a
    TensorTensor with 2 was rejected).  Hoist all but `max_waits` of each
    instruction's waits onto same-engine NOPs inserted right before it —
    engines execute in order, so semantics are unchanged."""
    idx = 0
    for fn in nc.m.functions:
        for bb in fn.blocks:
            new = []
            changed = False
            for inst in bb.instructions:
                si = inst.sync_info
                waits = list(si.on_wait) if si is not None and si.on_wait else []
                if len(waits) > max_waits:
                    changed = True
                    for w in waits[:-max_waits]:
                        idx += 1
                        nop = mybir.InstNoOp(
                            name=f"I-waitsplit-{idx}", engine=inst.engine)
                        nop.sync_info = bass_rust.SyncInfo(
                            on_wait=[w], on_update=[])
                        new.append(nop)
                    upd = list(si.on_update) if si.on_update else []
                    inst.sync_info = bass_rust.SyncInfo(
                        on_wait=waits[-max_waits:], on_update=upd)
                new.append(inst)
            if changed:
                bb.instructions = new
# ---------------------------------------------------------------------------


def _emit_prologue(nc, tc, vt_d, vnat_d, consts, work, psum_small):
    """Emit fp32 instructions computing A (4 sbuf tiles [128, 512])."""
    eye = consts.tile([128, 128], FP, tag="eye")
    make_identity(nc, eye)
    eye2 = consts.tile([128, 128], FP, tag="eye2")
    nc.vector.tensor_scalar_mul(eye2, eye, 2.0)
    triu = consts.tile([128, 128], FP, tag="triu")
    make_upper_triangular(nc, triu, val=1.0, diag=False)
    # padcol: 1.0 at rows >= NV - 4*128 = 2 (pad rows of the last block)
    padcol = consts.tile([128, 1], FP, tag="padcol")
    nc.gpsimd.memset(padcol, 1.0)
    nc.gpsimd.affine_select(
        out=padcol, in_=padcol, compare_op=OP.is_ge, fill=0.0,
        base=-(NV - 4 * 128), pattern=[[0, 1]], channel_multiplier=1,
    )

    vt_sb = []
    for k in range(4):
        t = consts.tile([128, NP], FP, tag=f"vt{k}", name=f"vt{k}")
        nc.sync.dma_start(out=t, in_=vt_d[k * 128:(k + 1) * 128, :])
        vt_sb.append(t)
    vnat_sb = []
    for j in range(NB):
        t = consts.tile([128, S], FP, tag=f"vnat{j}", name=f"vnat{j}")
        nc.sync.dma_start(out=t, in_=vnat_d[j * 128:(j + 1) * 128, :])
        vnat_sb.append(t)

    # --- G = V^T V, upper block triangle only (row mi needs cols >= mi*128:
    #     diagonal blocks feed RT, strictly-upper blocks feed back-subst) ---
    g_sb = []
    for mi in range(NB):
        g = consts.tile([128, NP], FP, tag=f"g{mi}", name=f"g{mi}")
        n0 = mi * 128
        chunks = [(n0, min(512, NP - n0))]
        if NP - n0 > 512:
            chunks.append((n0 + 512, NP - n0 - 512))
        for c0, cw in chunks:
            g_ps = psum_small.tile([128, cw], FP, tag="med", name=f"gps{mi}_{c0}")
            for k in range(4):
                nc.tensor.matmul(
                    g_ps,
                    lhsT=vt_sb[k][:, mi * 128:(mi + 1) * 128],
                    rhs=vt_sb[k][:, c0:c0 + cw],
                    start=(k == 0), stop=(k == 3),
                )
            nc.vector.tensor_copy(g[:, c0:c0 + cw], g_ps)
        g_sb.append(g)

    # --- per-block Newton inversion of the diagonal blocks of R ---
    xrow = []   # X stored as 5 row tiles [128, 640] (lower block triangle)
    for k in range(NB):
        xrow.append(consts.tile([128, NP], FP, tag=f"xrow{k}",
                                name=f"xrow{k}"))
    cs = []     # C_b = X_bb^T, needed for back-substitution
    for b in range(NB):
        sq = work.tile([128, S], FP, tag="sq")
        nc.vector.tensor_mul(sq, vnat_sb[b], vnat_sb[b])
        ss = work.tile([128, 1], FP, tag="ss")
        nc.vector.reduce_sum(ss, sq, axis=AX.X)
        rd = work.tile([128, 1], FP, tag="rd")
        # rd = (ss + EPS) * 0.5  (+1.0 on pad rows)
        nc.vector.tensor_scalar(rd, ss, EPS, 0.5, OP.add, OP.mult)
        if b == NB - 1:
            nc.vector.tensor_add(rd, rd, padcol)
        rinv = work.tile([128, 1], FP, tag="rinv")
        nc.vector.reciprocal(rinv, rd)

        # RT holds R_bb^T = striu(G_bb) + diag(rd)
        rt = work.tile([128, 128], FP, tag="rt")
        nc.vector.tensor_mul(rt, g_sb[b][:, b * 128:(b + 1) * 128], triu)
        nc.vector.scalar_tensor_tensor(
            out=rt, in0=eye, scalar=rd, in1=rt, op0=OP.mult, op1=OP.add)

        # X0 = C0 = diag(1/rd)
        x_cur = work.tile([128, 128], FP, tag="xn")
        nc.vector.tensor_scalar_mul(x_cur, eye, rinv)
        c_cur = x_cur
        for _ in range(NEWTON_ITERS):
            m1_ps = psum_small.tile([128, 128], FP, tag="pp")
            nc.tensor.matmul(m1_ps, lhsT=rt, rhs=x_cur,
                             start=True, stop=True)
            m2 = work.tile([128, 128], FP, tag="m2")
            # m2 = 2I - m1
            nc.vector.scalar_tensor_tensor(
                out=m2, in0=m1_ps, scalar=-1.0, in1=eye2,
                op0=OP.mult, op1=OP.add)
            xn_ps = psum_small.tile([128, 128], FP, tag="pp")
            nc.tensor.matmul(xn_ps, lhsT=c_cur, rhs=m2,
                             start=True, stop=True)
            cn_ps = psum_small.tile([128, 128], FP, tag="pp")
            nc.tensor.matmul(cn_ps, lhsT=m2, rhs=c_cur,
                             start=True, stop=True)
            x_new = work.tile([128, 128], FP, tag="xn")
            nc.vector.tensor_copy(x_new, xn_ps)
            c_new = work.tile([128, 128], FP, tag="cn")
            nc.vector.tensor_copy(c_new, cn_ps)
            x_cur, c_cur = x_new, c_new
        nc.vector.tensor_copy(xrow[b][:, b * 128:(b + 1) * 128], x_cur)
        c_keep = consts.tile([128, 128], FP, tag=f"c{b}", name=f"c{b}")
        nc.vector.tensor_copy(c_keep, c_cur)
        cs.append(c_keep)

    # --- off-diagonal blocks of X = R^{-1} via block back-substitution ---
    # X_ij = -X_ii (sum_{k=j..i-1} R_ik X_kj);  R_ik^T = G_ki (G symmetric)
    for j in range(NB):
        for i in range(j + 1, NB):
            acc_ps = psum_small.tile([128, 128], FP, tag="pp")
            for k in range(j, i):
                nc.tensor.matmul(
                    acc_ps,
                    lhsT=g_sb[k][:, i * 128:(i + 1) * 128],
                    rhs=xrow[k][:, j * 128:(j + 1) * 128],
                    start=(k == j), stop=(k == i - 1),
                )
            negacc = work.tile([128, 128], FP, tag="negacc")
            nc.scalar.mul(negacc, acc_ps, -1.0)
            xij_ps = psum_small.tile([128, 128], FP, tag="pp")
            nc.tensor.matmul(xij_ps, lhsT=cs[i], rhs=negacc,
                             start=True, stop=True)
            nc.vector.tensor_copy(xrow[i][:, j * 128:(j + 1) * 128], xij_ps)

    # --- WT_j = sum_{k>=j} X_kj^T vnat_k  (WT = (V T)^T, 5 tiles [128,512])
    wt_sb = []
    for j in range(NB):
        wt_ps = psum_small.tile([128, S], FP, tag="med", name=f"wtps{j}")
        for k in range(j, NB):
            nc.tensor.matmul(
                wt_ps,
                lhsT=xrow[k][:, j * 128:(j + 1) * 128],
                rhs=vnat_sb[k],
                start=(k == j), stop=(k == NB - 1),
            )
        wt = consts.tile([128, S], FP, tag=f"wt{j}", name=f"wt{j}")
        nc.vector.tensor_copy(wt, wt_ps)
        wt_sb.append(wt)

    # --- A = I - WT^T vnat  (4 tiles [128, 512], layout [s, s']) ---
    a_sb = []
    for st in range(4):
        a_ps = psum_small.tile([128, S], FP, tag="med", name=f"aps{st}")
        for j in range(NB):
            nc.tensor.matmul(
                a_ps,
                lhsT=wt_sb[j][:, st * 128:(st + 1) * 128],
                rhs=vnat_sb[j],
                start=(j == 0), stop=(j == NB - 1),
            )
        a = consts.tile([128, S], FP, tag=f"a{st}", name=f"a{st}")
        nc.scalar.mul(a, a_ps, -1.0)
        nc.vector.tensor_add(a[:, st * 128:(st + 1) * 128],
                             a[:, st * 128:(st + 1) * 128], eye)
        a_sb.append(a)
    return a_sb


def _emit_main_plain(nc, consts, xpool, ypool, psum_y, xt_d, y_d, a_sb):
    """Single-pass f32r main loop: 4 matmuls per 128-row output tile."""
    # provenance copies: f32r matmul operands must be produced as float32r
    a_r = []
    for k in range(4):
        ar = consts.tile([128, S], FPR, tag=f"ar{k}", name=f"ar{k}")
        nc.vector.tensor_copy(ar, a_sb[k])
        a_r.append(ar)

    for c in range(BPC // CW):
        xc = []
        for k in range(4):
            t32 = xpool.tile([128, CW], FP, tag=f"xc32_{k}")
            nc.sync.dma_start(
                out=t32, in_=xt_d[k * 128:(k + 1) * 128, c * CW:(c + 1) * CW])
            t = xpool.tile([128, CW], FPR, tag=f"xc{k}")
            nc.vector.tensor_copy(t, t32)
            xc.append(t)
        for bt in range(CW // 128):
            y_ps = psum_y.tile([128, S], FP, tag="y_ps")
            for k in range(4):
                nc.tensor.matmul(
                    y_ps,
                    lhsT=xc[k][:, bt * 128:(bt + 1) * 128],
                    rhs=a_r[k],
                    start=(k == 0), stop=(k == 3),
                )
            yt = ypool.tile([128, S], FP, tag="yt")
            nc.scalar.copy(yt, y_ps)
            row0 = (c * (CW // 128) + bt) * 128
            nc.sync.dma_start(out=y_d[row0:row0 + 128, :], in_=yt)


def _emit_main_compensated(nc, consts, xpool, ypool, psum_y, xt_d, y_d, a_sb):
    """3-term compensated main loop: y = xh Ah + xl Ah + xh Al."""
    a_h = []
    a_l = []
    for k in range(4):
        ah32 = consts.tile([128, S], FP, tag=f"ah32_{k}", name=f"ah32_{k}")
        nc.vector.tensor_scalar(
            ah32.bitcast(U32), a_sb[k].bitcast(U32), HEAD_MASK, None,
            OP.bitwise_and)
        ah = consts.tile([128, S], FPR, tag=f"ah{k}", name=f"ah{k}")
        nc.vector.tensor_copy(ah, ah32)
        al = consts.tile([128, S], FPR, tag=f"al{k}", name=f"al{k}")
        nc.vector.tensor_sub(al, a_sb[k], ah32)
        a_h.append(ah)
        a_l.append(al)

    for c in range(BPC // CW):
        xh = []
        xl = []
        for k in range(4):
            t32 = xpool.tile([128, CW], FP, tag=f"xc32_{k}")
            nc.sync.dma_start(
                out=t32, in_=xt_d[k * 128:(k + 1) * 128, c * CW:(c + 1) * CW])
            th32 = xpool.tile([128, CW], FP, tag=f"xh32_{k}")
            nc.vector.tensor_scalar(
                th32.bitcast(U32), t32.bitcast(U32), HEAD_MASK, None,
                OP.bitwise_and)
            th = xpool.tile([128, CW], FPR, tag=f"xh{k}")
            nc.vector.tensor_copy(th, th32)
            tl = xpool.tile([128, CW], FPR, tag=f"xl{k}")
            nc.scalar.activation(  # tl = t32 - th32, on ACT to offload DVE
                tl, th32, mybir.ActivationFunctionType.Copy,
                bias=0.0, scale=-1.0)
            nc.vector.tensor_add(tl, tl, t32)
            xh.append(th)
            xl.append(tl)
        for bt in range(CW // 128):
            y_ps = psum_y.tile([128, S], FP, tag="y_ps")
            bs = slice(bt * 128, (bt + 1) * 128)
            for k in range(4):
                nc.tensor.matmul(y_ps, lhsT=xh[k][:, bs], rhs=a_h[k],
                                 start=(k == 0), stop=False)
            for k in range(4):
                nc.tensor.matmul(y_ps, lhsT=xl[k][:, bs], rhs=a_h[k],
                                 start=False, stop=False)
            for k in range(4):
                nc.tensor.matmul(y_ps, lhsT=xh[k][:, bs], rhs=a_l[k],
                                 start=False, stop=(k == 3))
            yt = ypool.tile([128, S], FP, tag="yt")
            nc.scalar.copy(yt, y_ps)
            row0 = (c * (CW // 128) + bt) * 128
            nc.sync.dma_start(out=y_d[row0:row0 + 128, :], in_=yt)


def build_program(compensated=COMPENSATED, trace_sim=False):
    nc = bass.Bass("TRN2")
    xt_d = nc.dram_tensor("xt", [S, BPC], FP, kind="ExternalInput")
    vt_d = nc.dram_tensor("vt", [S, NP], FP, kind="ExternalInput")
    vnat_d = nc.dram_tensor("vnat", [NP, S], FP, kind="ExternalInput")
    y_d = nc.dram_tensor("y", [BPC, S], FP, kind="ExternalOutput")

    with tile.TileContext(nc, trace_sim=trace_sim) as tc, ExitStack() as ctx:
        consts = ctx.enter_context(tc.tile_pool(name="consts", bufs=1))
        work = ctx.enter_context(tc.tile_pool(name="work", bufs=3))
        xpool = ctx.enter_context(tc.tile_pool(name="xpool", bufs=3))
        ypool = ctx.enter_context(tc.tile_pool(name="ypool", bufs=4))
        psum_small = ctx.enter_context(
            tc.tile_pool(name="psum_small", bufs=2, space="PSUM"))
        psum_y = ctx.enter_context(
            tc.tile_pool(name="psum_y", bufs=4, space="PSUM"))

        a_sb = _emit_prologue(nc, tc, vt_d, vnat_d, consts, work, psum_small)
        if compensated:
            _emit_main_compensated(nc, consts, xpool, ypool, psum_y,
                                   xt_d, y_d, a_sb)
        else:
            _emit_main_plain(nc, consts, xpool, ypool, psum_y,
                             xt_d, y_d, a_sb)
    _split_excess_waits(nc)
    return nc


_NC_CACHE = {}


def _get_nc():
    if "nc" not in _NC_CACHE:
        _NC_CACHE["nc"] = build_program()
    return _NC_CACHE["nc"]


def prepare_in_maps(x, vectors):
    x = np.ascontiguousarray(np.asarray(x, dtype=np.float32))
    v = np.asarray(vectors, dtype=np.float32)[..., 0]  # [514, 512]
    vnat = np.zeros((NP, S), np.float32)
    vnat[:NV] = v
    vt = np.ascontiguousarray(vnat.T)                  # [512, 640]
    xt = np.ascontiguousarray(x.T)                     # [512, 65536]
    in_maps = []
    for c in range(NCORES):
        in_maps.append({
            "xt": np.ascontiguousarray(xt[:, c * BPC:(c + 1) * BPC]),
            "vt": vt,
            "vnat": vnat,
        })
    return in_maps


def kernel(x, vectors):
    nc = _get_nc()
    in_maps = prepare_in_maps(x, vectors)
    res = run_bass_kernel_spmd(nc, in_maps, list(range(NCORES)))
    y = np.concatenate([r["y"] for r in res.results], axis=0)
    return np.ascontiguousarray(y.astype(np.float32))


if __name__ == "__main__":
    rng = np.random.default_rng(0)
    x = rng.standard_normal((B, S)).astype(np.float32)
    v = rng.standard_normal((NV, S, 1)).astype(np.float32)
    v /= np.linalg.norm(v, axis=1, keepdims=True)
    y = kernel(x, v)
    print("y", y.shape, y.dtype, float(np.abs(y).max()))



# revision 2
# speedup vs baseline: 2.6754x; 2.6754x over previous
"""Trainium2 Bass kernel for the Householder-chain problem.

Computes y = x @ Q.T where Q = M_0 @ M_1 @ ... @ M_{N-1} is a product of
N=514 Householder reflections M_i = I - 2 v_i v_i^T / (v_i^T v_i + eps)
over S=512 dims, and x is [65536, 512].

Math: since each M_i is symmetric, Q.T = M_{N-1} @ ... @ M_0 =: A, and the
product collapses via the compact-WY representation with natural column
order:  A = I - V T V^T  where V = [v_0 ... v_{N-1}] (S x N) and
T^{-1} = R = stril(V^T V) + diag((||v_i||^2 + eps)/2)   (lower triangular).

A is tiny (512 x 512) and depends only on `vectors`, so it is computed
once on the host in float64 (exact to ~1e-15; the end-to-end error budget
is set by bf16 rounding below, ~3e-3 against a 2e-2 gate).

Device work is the single streaming matmul y = x @ A, data-parallel over
the 65536 rows across 8 cores (8192 rows/core).  Everything on device is
bf16 (fp32 PSUM accumulation): bf16 halves HBM traffic vs fp32 and runs
the PE at 1 column/cycle with hidden fast-weight-loads (the fp32/f32r
paths are 2-4x slower on both axes).  x is transposed on the host once so
the contraction dim (s) lands on SBUF partitions.
"""

from contextlib import ExitStack

import ml_dtypes
import numpy as np

import bass_rust
import concourse.bass as bass
import concourse.mybir as mybir
import concourse.tile as tile
from concourse.bass_utils import run_bass_kernel_spmd
from concourse.vector_clock import ScopedClock

FP = mybir.dt.float32
BF = mybir.dt.bfloat16

S = 512           # feature dim
NV = 514          # number of householder vectors
B = 65536         # batch rows
NCORES = 8
BPC = B // NCORES  # 8192 rows per core
EPS = 1e-16
CW = 2048         # main-loop x chunk width (batch cols per chunk)
BF_NP = ml_dtypes.bfloat16


# ---------------------------------------------------------------------------
# walrus CTRL instructions accept at most 4 sem waits, and this Tile
# version puts the whole global-clock wait set on the single tail drain.
# Spread the waits over preceding SP nops (1 wait each, conservatively).
def _patched_drain_and_barrier(self, tick_clock, wait_clock):
    pre_nops = [self.nc.sync.nop() for _ in range(30)]
    drain_inst = self.nc.sync.drain()
    wait_clock.add_sem_waits(
        drain_inst.ins, ScopedClock({None: tick_clock.global_clock})
    )
    si = drain_inst.ins.sync_info
    waits = list(si.on_wait) if si is not None and si.on_wait else []
    if len(waits) > 1:
        assert len(waits) - 1 <= len(pre_nops), "too many drain waits"
        for nop, w in zip(pre_nops, waits[:-1]):
            nop.ins.sync_info = bass_rust.SyncInfo(on_wait=[w], on_update=[])
        upd = list(si.on_update) if si.on_update else []
        drain_inst.ins.sync_info = bass_rust.SyncInfo(
            on_wait=[waits[-1]], on_update=upd)

    self.nc.all_engine_barrier()
    assert self.sems is not None
    popped = self.nc._tile_sem_poison_stack.pop()
    assert popped is self._sem_poison
    self.nc.clear_and_free_semaphores(list(self.sems.allocated().values()))
    self.nc.all_engine_barrier()


tile.TileContext._drain_and_barrier = _patched_drain_and_barrier


def _split_excess_waits(nc, max_waits=1):
    """This walrus build accepts very few sem waits per instruction (a
    TensorTensor with 2 was rejected).  Hoist all but `max_waits` of each
    instruction's waits onto same-engine NOPs inserted right before it —
    engines execute in order, so semantics are unchanged."""
    idx = 0
    for fn in nc.m.functions:
        for bb in fn.blocks:
            new = []
            changed = False
            for inst in bb.instructions:
                si = inst.sync_info
                waits = list(si.on_wait) if si is not None and si.on_wait else []
                if len(waits) > max_waits:
                    changed = True
                    for w in waits[:-max_waits]:
                        idx += 1
                        nop = mybir.InstNoOp(
                            name=f"I-waitsplit-{idx}", engine=inst.engine)
                        nop.sync_info = bass_rust.SyncInfo(
                            on_wait=[w], on_update=[])
                        new.append(nop)
                    upd = list(si.on_update) if si.on_update else []
                    inst.sync_info = bass_rust.SyncInfo(
                        on_wait=waits[-max_waits:], on_update=upd)
                new.append(inst)
            if changed:
                bb.instructions = new
# ---------------------------------------------------------------------------


def build_program(trace_sim=False):
    nc = bass.Bass("TRN2")
    xt_d = nc.dram_tensor("xt", [S, BPC], BF, kind="ExternalInput")
    a_d = nc.dram_tensor("a", [S, S], BF, kind="ExternalInput")
    y_d = nc.dram_tensor("y", [BPC, S], BF, kind="ExternalOutput")

    with tile.TileContext(nc, trace_sim=trace_sim) as tc, ExitStack() as ctx:
        consts = ctx.enter_context(tc.tile_pool(name="consts", bufs=1))
        xpool = ctx.enter_context(tc.tile_pool(name="xpool", bufs=3))
        ypool = ctx.enter_context(tc.tile_pool(name="ypool", bufs=8))
        psum_y = ctx.enter_context(
            tc.tile_pool(name="psum_y", bufs=4, space="PSUM"))

        a_sb = []
        for k in range(4):
            t = consts.tile([128, S], BF, tag=f"a{k}", name=f"a{k}")
            nc.sync.dma_start(out=t, in_=a_d[k * 128:(k + 1) * 128, :])
            a_sb.append(t)

        for c in range(BPC // CW):
            xc = []
            for k in range(4):
                t = xpool.tile([128, CW], BF, tag=f"xc{k}")
                nc.sync.dma_start(
                    out=t,
                    in_=xt_d[k * 128:(k + 1) * 128, c * CW:(c + 1) * CW])
                xc.append(t)
            for bt in range(CW // 128):
                y_ps = psum_y.tile([128, S], FP, tag="y_ps")
                for k in range(4):
                    nc.tensor.matmul(
                        y_ps,
                        lhsT=xc[k][:, bt * 128:(bt + 1) * 128],
                        rhs=a_sb[k],
                        start=(k == 0), stop=(k == 3),
                    )
                yt = ypool.tile([128, S], BF, tag="yt")
                # alternate PSUM-drain engines so neither becomes the
                # bottleneck behind the PE
                if bt % 2 == 0:
                    nc.scalar.copy(yt, y_ps)
                else:
                    nc.vector.tensor_copy(yt, y_ps)
                row0 = (c * (CW // 128) + bt) * 128
                nc.sync.dma_start(out=y_d[row0:row0 + 128, :], in_=yt)
    _split_excess_waits(nc)
    return nc


_NC_CACHE = {}


def _get_nc():
    if "nc" not in _NC_CACHE:
        _NC_CACHE["nc"] = build_program()
    return _NC_CACHE["nc"]


def _compute_A(vectors):
    """A = Q^T = I - V R^{-1} V^T in float64 on the host."""
    v = np.asarray(vectors, np.float64)[..., 0]        # [N, S]
    V = v.T                                            # [S, N]
    G = v @ V                                          # [N, N] = V^T V
    R = np.tril(G, -1) + np.diag((np.einsum("ns,ns->n", v, v) + EPS) / 2.0)
    Z = np.linalg.solve(R, V.T)                        # [N, S] = T V^T
    A = np.eye(S) - V @ Z                              # [S, S] = Q^T
    return A


def prepare_in_maps(x, vectors):
    a_bf = np.ascontiguousarray(_compute_A(vectors).astype(BF_NP))
    xb = np.asarray(x, dtype=np.float32).astype(BF_NP)  # [B, S] bf16
    in_maps = []
    for c in range(NCORES):
        xt = np.ascontiguousarray(xb[c * BPC:(c + 1) * BPC].T)  # [S, BPC]
        in_maps.append({"xt": xt, "a": a_bf})
    return in_maps


def kernel(x, vectors):
    nc = _get_nc()
    in_maps = prepare_in_maps(x, vectors)
    res = run_bass_kernel_spmd(nc, in_maps, list(range(NCORES)))
    y = np.concatenate([r["y"] for r in res.results], axis=0)
    return np.ascontiguousarray(y.astype(np.float32))


if __name__ == "__main__":
    rng = np.random.default_rng(0)
    x = rng.standard_normal((B, S)).astype(np.float32)
    v = rng.standard_normal((NV, S, 1)).astype(np.float32)
    v /= np.linalg.norm(v, axis=1, keepdims=True)
    y = kernel(x, v)
    print("y", y.shape, y.dtype, float(np.abs(y).max()))


# revision 5
# speedup vs baseline: 3.1924x; 1.1932x over previous
"""Trainium2 Bass kernel for the Householder-chain problem.

Computes y = x @ Q.T where Q = M_0 @ M_1 @ ... @ M_{N-1} is a product of
N=514 Householder reflections M_i = I - 2 v_i v_i^T / (v_i^T v_i + eps)
over S=512 dims, and x is [65536, 512].

Math: since each M_i is symmetric, Q.T = M_{N-1} @ ... @ M_0 =: A, and the
product collapses via the compact-WY representation with natural column
order:  A = I - V T V^T  where V = [v_0 ... v_{N-1}] (S x N) and
T^{-1} = R = stril(V^T V) + diag((||v_i||^2 + eps)/2)   (lower triangular).

A is tiny (512 x 512) and depends only on `vectors`, so it is computed
once on the host in float64 (exact to ~1e-15; the end-to-end error budget
is set by bf16 rounding below, ~3e-3 against a 2e-2 gate).

Device work is the single streaming matmul y = x @ A, data-parallel over
the 65536 rows across 8 cores (8192 rows/core), all in bf16 with fp32
PSUM accumulation (bf16 halves HBM traffic vs fp32 and runs the PE at
1 column/cycle with hidden fast-weight-loads).

Layouts are packed on the host so every DMA is one large contiguous
transfer: x arrives as per-chunk [128, 4*w] blocks (x^T tiles for the 4
contraction sub-blocks side by side), y leaves in groups of 8 row-tiles
as [128, 8*512] blocks (~1 MiB per DMA).  The first chunks are small so
the PE starts within a few microseconds, and a handful of throwaway
matmuls at t=0 trip the PE clock-gate (HAM) to full rate while the first
DMAs are still in flight.
"""

from contextlib import ExitStack

import ml_dtypes
import numpy as np

import bass_rust
import concourse.bass as bass
import concourse.mybir as mybir
import concourse.tile as tile
from concourse.bass_utils import run_bass_kernel_spmd
from concourse.vector_clock import ScopedClock

FP = mybir.dt.float32
BF = mybir.dt.bfloat16

S = 512           # feature dim
NV = 514          # number of householder vectors
B = 65536         # batch rows
NCORES = 8
BPC = B // NCORES  # 8192 rows per core
EPS = 1e-16
BF_NP = ml_dtypes.bfloat16

CHUNKS = [512, 512, 1024, 2048, 2048, 2048]   # batch cols per x chunk
assert sum(CHUNKS) == BPC
YGRP = 8          # y row-tiles per output DMA (8 * 128 rows = 1 MiB bf16)
NGRP = BPC // (128 * YGRP)
WARM_MM = 7       # PE prewarm matmuls during initial DMA wait


# ---------------------------------------------------------------------------
# walrus CTRL instructions accept at most 4 sem waits, and this Tile
# version puts the whole global-clock wait set on the single tail drain.
# Spread the waits over preceding SP nops (1 wait each, conservatively).
def _patched_drain_and_barrier(self, tick_clock, wait_clock):
    pre_nops = [self.nc.sync.nop() for _ in range(30)]
    drain_inst = self.nc.sync.drain()
    wait_clock.add_sem_waits(
        drain_inst.ins, ScopedClock({None: tick_clock.global_clock})
    )
    si = drain_inst.ins.sync_info
    waits = list(si.on_wait) if si is not None and si.on_wait else []
    if len(waits) > 1:
        assert len(waits) - 1 <= len(pre_nops), "too many drain waits"
        for nop, w in zip(pre_nops, waits[:-1]):
            nop.ins.sync_info = bass_rust.SyncInfo(on_wait=[w], on_update=[])
        upd = list(si.on_update) if si.on_update else []
        drain_inst.ins.sync_info = bass_rust.SyncInfo(
            on_wait=[waits[-1]], on_update=upd)

    self.nc.all_engine_barrier()
    assert self.sems is not None
    popped = self.nc._tile_sem_poison_stack.pop()
    assert popped is self._sem_poison
    self.nc.clear_and_free_semaphores(list(self.sems.allocated().values()))
    self.nc.all_engine_barrier()


tile.TileContext._drain_and_barrier = _patched_drain_and_barrier


def _split_excess_waits(nc, max_waits=1):
    """This walrus build accepts very few sem waits per instruction (a
    TensorTensor with 2 was rejected).  Hoist all but `max_waits` of each
    instruction's waits onto same-engine NOPs inserted right before it —
    engines execute in order, so semantics are unchanged."""
    idx = 0
    for fn in nc.m.functions:
        for bb in fn.blocks:
            new = []
            changed = False
            for inst in bb.instructions:
                si = inst.sync_info
                waits = list(si.on_wait) if si is not None and si.on_wait else []
                if len(waits) > max_waits:
                    changed = True
                    for w in waits[:-max_waits]:
                        idx += 1
                        nop = mybir.InstNoOp(
                            name=f"I-waitsplit-{idx}", engine=inst.engine)
                        nop.sync_info = bass_rust.SyncInfo(
                            on_wait=[w], on_update=[])
                        new.append(nop)
                    upd = list(si.on_update) if si.on_update else []
                    inst.sync_info = bass_rust.SyncInfo(
                        on_wait=waits[-max_waits:], on_update=upd)
                new.append(inst)
            if changed:
                bb.instructions = new
# ---------------------------------------------------------------------------


def build_program(trace_sim=False):
    nc = bass.Bass("TRN2")
    xc_d = [
        nc.dram_tensor(f"xc{ci}", [128, 4 * w], BF, kind="ExternalInput")
        for ci, w in enumerate(CHUNKS)
    ]
    a_d = nc.dram_tensor("a", [128, 4 * S], BF, kind="ExternalInput")
    y_d = nc.dram_tensor("y", [NGRP * 128, YGRP * S], BF,
                         kind="ExternalOutput")

    with tile.TileContext(nc, trace_sim=trace_sim) as tc, ExitStack() as ctx:
        consts = ctx.enter_context(tc.tile_pool(name="consts", bufs=1))
        xpool = ctx.enter_context(tc.tile_pool(name="xpool", bufs=3))
        ypool = ctx.enter_context(tc.tile_pool(name="ypool", bufs=3))
        psum_y = ctx.enter_context(
            tc.tile_pool(name="psum_y", bufs=4, space="PSUM"))

        # PE prewarm: throwaway matmuls with no DMA dependencies trip the
        # HAM clock gate to 2.4 GHz while the first x/a loads are in
        # flight.  Results are never read.
        warm = consts.tile([128, S], BF, tag="warm")
        nc.gpsimd.memset(warm, 0.0)
        for i in range(WARM_MM):
            w_ps = psum_y.tile([128, S], FP, tag="y_ps")
            nc.tensor.matmul(w_ps, lhsT=warm[:, :128], rhs=warm,
                             start=True, stop=True)

        a_t = consts.tile([128, 4 * S], BF, tag="a")
        nc.sync.dma_start(out=a_t, in_=a_d[:, :])

        gt = 0          # global output row-tile index
        ybuf = None
        for ci, w in enumerate(CHUNKS):
            xc = xpool.tile([128, 4 * w], BF, tag=f"xc{w}")
            nc.sync.dma_start(out=xc, in_=xc_d[ci][:, :])
            for bt in range(w // 128):
                y_ps = psum_y.tile([128, S], FP, tag="y_ps")
                for k in range(4):
                    nc.tensor.matmul(
                        y_ps,
                        lhsT=xc[:, k * w + bt * 128:k * w + (bt + 1) * 128],
                        rhs=a_t[:, k * S:(k + 1) * S],
                        start=(k == 0), stop=(k == 3),
                    )
                g, slot = divmod(gt, YGRP)
                if slot == 0:
                    ybuf = ypool.tile([128, YGRP * S], BF, tag="ybuf")
                # alternate PSUM-drain engines so neither becomes the
                # bottleneck behind the PE
                dst = ybuf[:, slot * S:(slot + 1) * S]
                if gt % 2 == 0:
                    nc.scalar.copy(dst, y_ps)
                else:
                    nc.vector.tensor_copy(dst, y_ps)
                if slot == YGRP - 1:
                    nc.sync.dma_start(
                        out=y_d[g * 128:(g + 1) * 128, :], in_=ybuf)
                gt += 1
    _split_excess_waits(nc)
    return nc


_NC_CACHE = {}


def _get_nc():
    if "nc" not in _NC_CACHE:
        _NC_CACHE["nc"] = build_program()
    return _NC_CACHE["nc"]


def _compute_A(vectors):
    """A = Q^T = I - V R^{-1} V^T in float64 on the host."""
    v = np.asarray(vectors, np.float64)[..., 0]        # [N, S]
    V = v.T                                            # [S, N]
    G = v @ V                                          # [N, N] = V^T V
    R = np.tril(G, -1) + np.diag((np.einsum("ns,ns->n", v, v) + EPS) / 2.0)
    Z = np.linalg.solve(R, V.T)                        # [N, S] = T V^T
    A = np.eye(S) - V @ Z                              # [S, S] = Q^T
    return A


def prepare_in_maps(x, vectors):
    # a[p, k*512 + col] = A[k*128 + p, col]
    A = _compute_A(vectors).astype(np.float32).astype(BF_NP)
    a_pk = np.ascontiguousarray(
        A.reshape(4, 128, S).transpose(1, 0, 2).reshape(128, 4 * S))
    xb = np.asarray(x, dtype=np.float32).astype(BF_NP)  # [B, S] bf16
    in_maps = []
    for c in range(NCORES):
        xcore = xb[c * BPC:(c + 1) * BPC]               # [BPC, S]
        m = {"a": a_pk}
        b0 = 0
        for ci, w in enumerate(CHUNKS):
            # xc[p, k*w + col] = x^T[k*128 + p, b0 + col]
            blk = xcore[b0:b0 + w].reshape(w, 4, 128)   # [col, k, p]
            m[f"xc{ci}"] = np.ascontiguousarray(
                blk.transpose(2, 1, 0).reshape(128, 4 * w))
            b0 += w
        in_maps.append(m)
    return in_maps


def _unpack_y(yarr):
    # yarr[g*128 + p, t*512 + col] -> y[g*1024 + t*128 + p, col]
    return (yarr.reshape(NGRP, 128, YGRP, S)
            .transpose(0, 2, 1, 3).reshape(BPC, S))


def kernel(x, vectors):
    nc = _get_nc()
    in_maps = prepare_in_maps(x, vectors)
    res = run_bass_kernel_spmd(nc, in_maps, list(range(NCORES)))
    y = np.concatenate([_unpack_y(r["y"]) for r in res.results], axis=0)
    return np.ascontiguousarray(y.astype(np.float32))


if __name__ == "__main__":
    rng = np.random.default_rng(0)
    x = rng.standard_normal((B, S)).astype(np.float32)
    v = rng.standard_normal((NV, S, 1)).astype(np.float32)
    v /= np.linalg.norm(v, axis=1, keepdims=True)
    y = kernel(x, v)
    print("y", y.shape, y.dtype, float(np.abs(y).max()))
